# revision 1
# baseline (speedup 1.0000x reference)
"""MetaPathAggregator kernel for Trainium2 (8 NeuronCores, data-parallel).

Math: the reference module is linear in the four gathered feature rows:

    dis  = 0.125*(mi+g1)@Wdd^T + 0.25*g2 + 0.5*dr
    drug = 0.125*(dr+g2)@Wdg^T + 0.25*g1 + 0.5*mi
    out  = [drug @ Wdrug^T | dis @ Wdis^T]
         = mi@M_mi + g1@M_g1 + g2@M_g2 + dr@M_dr

with per-slot 128x128 matrices

    M_mi = [0.500*C | 0.125*A]      A = Wdd^T @ Wdis^T   (128x64)
    M_g1 = [0.250*C | 0.125*A]      B = Wdg^T @ Wdrug^T  (128x64)
    M_g2 = [0.125*B | 0.250*D]      C = Wdrug^T          (128x64)
    M_dr = [0.125*B | 0.500*D]      D = Wdis^T           (128x64)

Since mp_ins indices are < 1000 (spec fill_max), only the first 1024 rows of
each feature table are live.  The kernel transforms the tables once on-device
(T_x = feat_x @ M_x, PE matmuls) and the per-token work collapses to four
row-gathers and three adds: out[t] = T_mi[i0]+T_g1[i1]+T_g2[i2]+T_dr[i3].

Device schedule per core (16384 tokens): prep (weights -> M matrices -> T
tables in DRAM scratch), then 16 chunks x (4 dma_gather of 1024 rows + 3 DVE
adds + 1 streaming store), with gene-table gathers software-pipelined behind
the mi/dr gathers.  HBM traffic/core ~46MB => memory(HBM-BW)-bound; the
TimelineSim cost model puts the schedule within ~10us of that roofline.
"""

import numpy as np

P = 128          # partitions
F = 128          # input feature dim
H = 128          # output hidden dim
HH = 64          # half hidden
R = 1024         # padded table rows (indices < 1000)
N_CORES = 8
B_PAIRS = 1024
BAG = 128
TOK = B_PAIRS * BAG // N_CORES   # 16384 tokens per core
CH = 1024                        # tokens per chunk (1024 descs per dma_gather)
NCH = TOK // CH                  # 16 chunks
CPB = CH // P                    # 8 tokens per partition per chunk

_CACHE = {}


def _build_module(do_gathers=True, do_adds=True, do_stores=True):
    import concourse.bacc as bacc
    import concourse.mybir as mybir
    import concourse.tile as tile
    from concourse.masks import make_identity
    from concourse.tile_rust import add_dep_helper

    f32 = mybir.dt.float32
    i16 = mybir.dt.int16

    nc = bacc.Bacc("TRN2", dynamic_dma_scratch_size=65536)

    feat_in = {
        "mi": nc.dram_tensor("feat_mi", [R, F], f32, kind="ExternalInput"),
        "ge": nc.dram_tensor("feat_ge", [R, F], f32, kind="ExternalInput"),
        "dr": nc.dram_tensor("feat_dr", [R, F], f32, kind="ExternalInput"),
    }
    w_dd = nc.dram_tensor("w_dd", [H, F], f32, kind="ExternalInput")
    w_dg = nc.dram_tensor("w_dg", [H, F], f32, kind="ExternalInput")
    w_drug = nc.dram_tensor("w_drug", [HH, F], f32, kind="ExternalInput")
    w_dis = nc.dram_tensor("w_dis", [HH, F], f32, kind="ExternalInput")
    idx_in = nc.dram_tensor("idx", [P, 4, NCH, CH // 16], i16, kind="ExternalInput")
    out = nc.dram_tensor("out", [TOK, H], f32, kind="ExternalOutput")

    with tile.TileContext(nc) as tc:
        with (
            tc.tile_pool(name="const", bufs=1) as cpool,
            tc.tile_pool(name="prep", bufs=2) as ppool,
            tc.tile_pool(name="psum", bufs=2, space="PSUM") as pspool,
            tc.tile_pool(name="tdram", bufs=1, space="DRAM") as dpool,
            tc.tile_pool(name="gather", bufs=4) as gpool,
        ):
            ident = cpool.tile([P, P], f32)
            make_identity(nc, ident[:])

            idx_t = cpool.tile([P, 4, NCH, CH // 16], i16)
            nc.sync.dma_start(idx_t[:], idx_in[:, :, :, :])

            # ---- load weights
            wdd_t = cpool.tile([H, F], f32, tag="wdd")
            nc.sync.dma_start(wdd_t[:], w_dd[:, :])
            wdg_t = cpool.tile([H, F], f32, tag="wdg")
            nc.sync.dma_start(wdg_t[:], w_dg[:, :])
            wdrug_t = cpool.tile([HH, F], f32, tag="wdrug")
            nc.sync.dma_start(wdrug_t[:], w_drug[:, :])
            wdis_t = cpool.tile([HH, F], f32, tag="wdis")
            nc.sync.dma_start(wdis_t[:], w_dis[:, :])

            # ---- C = Wdrug^T, D = Wdis^T  (PE transpose via identity)
            c_ps = pspool.tile([F, HH], f32, tag="tps")
            nc.tensor.transpose(out=c_ps[:], in_=wdrug_t[:], identity=ident[:HH, :HH])
            c_s = cpool.tile([F, HH], f32, tag="c_s")
            nc.vector.tensor_copy(out=c_s[:], in_=c_ps[:])

            d_ps = pspool.tile([F, HH], f32, tag="tps")
            nc.tensor.transpose(out=d_ps[:], in_=wdis_t[:], identity=ident[:HH, :HH])
            d_s = cpool.tile([F, HH], f32, tag="d_s")
            nc.vector.tensor_copy(out=d_s[:], in_=d_ps[:])

            # ---- A = Wdd^T @ Wdis^T, B = Wdg^T @ Wdrug^T
            a_ps = pspool.tile([F, HH], f32, tag="abps")
            nc.tensor.matmul(out=a_ps[:], lhsT=wdd_t[:], rhs=d_s[:], start=True, stop=True)
            b_ps = pspool.tile([F, HH], f32, tag="abps")
            nc.tensor.matmul(out=b_ps[:], lhsT=wdg_t[:], rhs=c_s[:], start=True, stop=True)

            # ---- assemble M matrices [F, H] in SBUF
            m = {k: cpool.tile([F, H], f32, tag=f"m_{k}", name=f"m_{k}") for k in range(4)}
            # slot 0 = mi, 1 = g1, 2 = g2, 3 = dr
            nc.vector.tensor_scalar_mul(m[0][:, :HH], c_s[:], 0.5)
            nc.vector.tensor_scalar_mul(m[0][:, HH:], a_ps[:], 0.125)
            nc.vector.tensor_scalar_mul(m[1][:, :HH], c_s[:], 0.25)
            nc.vector.tensor_scalar_mul(m[1][:, HH:], a_ps[:], 0.125)
            nc.vector.tensor_scalar_mul(m[2][:, :HH], b_ps[:], 0.125)
            nc.vector.tensor_scalar_mul(m[2][:, HH:], d_s[:], 0.25)
            nc.vector.tensor_scalar_mul(m[3][:, :HH], b_ps[:], 0.125)
            nc.vector.tensor_scalar_mul(m[3][:, HH:], d_s[:], 0.5)

            # ---- transform tables: T_k = feat @ M_k  -> DRAM scratch
            NT = R // P  # 8 row-tiles per table
            t_dram = [dpool.tile([R, F], f32, tag=f"t{k}", name=f"t_dram{k}") for k in range(4)]
            t_store = [None] * 4  # store instruction per table (for gather deps)

            feat_tiles = {}
            for name, hbm in feat_in.items():
                ft = cpool.tile([P, NT, F], f32, tag=f"feat_{name}", name=f"feat_tile_{name}")
                nc.sync.dma_start(
                    ft[:], hbm[:, :].rearrange("(r p) f -> p r f", p=P)
                )
                feat_tiles[name] = ft

            # per feature table: transpose row-tiles, then transform every slot
            # that uses it (gene feeds both g1 and g2) and store to DRAM.
            # Ordered per table so early tables' gathers can start during prep.
            feat_slots = {"mi": [0], "ge": [1, 2], "dr": [3]}
            staged = {k: ppool.tile([P, NT, F], f32, tag=f"tstage{k}", name=f"tstage{k}")
                      for k in range(4)}
            for name in ("mi", "dr", "ge"):
                for r in range(NT):
                    tp = pspool.tile([P, P], f32, tag="ftps")
                    nc.tensor.transpose(
                        out=tp[:], in_=feat_tiles[name][:, r, :], identity=ident[:]
                    )
                    fts = ppool.tile([P, P], f32, tag="ftT", name=f"ftT_{name}_{r}",
                                     bufs=3)
                    # alternate PSUM->SBUF copies between DVE and ACT
                    if r % 2 == 0:
                        nc.vector.tensor_copy(out=fts[:], in_=tp[:])
                    else:
                        nc.scalar.activation(
                            out=fts[:], in_=tp[:],
                            func=mybir.ActivationFunctionType.Copy,
                        )
                    for k in feat_slots[name]:
                        mm = pspool.tile([P, H], f32, tag="mmps")
                        nc.tensor.matmul(
                            out=mm[:], lhsT=fts[:], rhs=m[k][:],
                            start=True, stop=True,
                        )
                        if k % 2 == 0:
                            nc.vector.tensor_copy(out=staged[k][:, r, :], in_=mm[:])
                        else:
                            nc.scalar.activation(
                                out=staged[k][:, r, :], in_=mm[:],
                                func=mybir.ActivationFunctionType.Copy,
                            )
                for k in feat_slots[name]:
                    t_store[k] = nc.sync.dma_start(
                        t_dram[k][:, :].rearrange("(r p) f -> p r f", p=P),
                        staged[k][:],
                    )

            # ---- main loop: gather + add + store
            # The Pool engine runs gathers in emission order.  The gene table
            # (slots 1,2) finishes its transform last, so its gathers are
            # delayed by GE_DELAY chunks relative to mi/dr gathers -- the
            # Pool engine streams ready mi/dr gathers instead of stalling at
            # the head of the queue waiting for the gene T table.
            GE_DELAY = 5
            gtiles = {}

            def issue_gather(k, ch):
                bufs = GE_DELAY + 2 if k in (0, 3) else 5
                gt = gpool.tile([P, CPB, F], f32, tag=f"g{k}", name=f"g{k}_{ch}",
                                bufs=bufs)
                if do_gathers:
                    gi = nc.gpsimd.dma_gather(
                        gt[:], t_dram[k][:, :], idx_t[:, k, ch, :], CH, CH, F,
                    )
                    add_dep_helper(gi.ins, t_store[k].ins, reason="gather after T store")
                gtiles[(k, ch)] = gt

            for ch in range(NCH + GE_DELAY):
                if ch < NCH:
                    issue_gather(0, ch)
                    issue_gather(3, ch)
                ch2 = ch - GE_DELAY
                if 0 <= ch2 < NCH:
                    issue_gather(1, ch2)
                    issue_gather(2, ch2)
                    g = [gtiles[(k, ch2)] for k in range(4)]
                    if do_adds:
                        nc.vector.tensor_add(g[0][:], g[0][:], g[1][:])
                        nc.vector.tensor_add(g[2][:], g[2][:], g[3][:])
                        nc.vector.tensor_add(g[0][:], g[0][:], g[2][:])
                    if do_stores:
                        nc.sync.dma_start(
                            out[ch2 * CH : (ch2 + 1) * CH, :].rearrange(
                                "(p s) h -> p s h", p=P),
                            g[0][:],
                        )

    nc.compile()
    return nc


def _prep_inputs(feat_miRNA, feat_gene, feat_drug, W_drug_disease, W_disease_drug,
                 W_drug, W_dis, mp_ins):
    """Marshal full inputs into per-core in_maps (no arithmetic on values)."""
    def pad_rows(a):
        a = np.ascontiguousarray(np.asarray(a, dtype=np.float32))
        if a.shape[0] >= R:
            return np.ascontiguousarray(a[:R])
        out = np.zeros((R, a.shape[1]), dtype=np.float32)
        out[: a.shape[0]] = a
        return out

    f_mi = pad_rows(feat_miRNA)
    f_ge = pad_rows(feat_gene)
    f_dr = pad_rows(feat_drug)
    wdd = np.ascontiguousarray(np.asarray(W_drug_disease, np.float32))
    wdg = np.ascontiguousarray(np.asarray(W_disease_drug, np.float32))
    wdrug = np.ascontiguousarray(np.asarray(W_drug, np.float32))
    wdis = np.ascontiguousarray(np.asarray(W_dis, np.float32))

    mp = np.asarray(mp_ins)
    assert mp.shape == (B_PAIRS, BAG, 4), mp.shape

    # gather-slot permutation: out[p, s] holds token p*CPB+s of the chunk;
    # gather slot j = s*128+p; wrapped idx layout: j -> [j%16, j//16], x8 groups
    j = np.arange(CH)
    tok_of_j = (j % P) * CPB + (j // P)          # token within chunk for slot j

    in_maps = []
    for core in range(N_CORES):
        mp_core = mp[core * (B_PAIRS // N_CORES) : (core + 1) * (B_PAIRS // N_CORES)]
        mp_core = mp_core.reshape(TOK, 4).astype(np.int16)
        idx_arr = np.empty((P, 4, NCH, CH // 16), dtype=np.int16)
        for ch in range(NCH):
            t = ch * CH + tok_of_j                 # absolute token per slot j
            for k in range(4):
                lin = mp_core[t, k]                # idx for gather slot j
                wrapped = lin.reshape(CH // 16, 16).T   # [16, CH/16]
                idx_arr[:, k, ch, :] = np.tile(wrapped, (8, 1))
        in_maps.append(
            {
                "feat_mi": f_mi,
                "feat_ge": f_ge,
                "feat_dr": f_dr,
                "w_dd": wdd,
                "w_dg": wdg,
                "w_drug": wdrug,
                "w_dis": wdis,
                "idx": idx_arr,
            }
        )
    return in_maps


def _numpy_fallback(feat_miRNA, feat_gene, feat_drug, W_drug_disease,
                    W_disease_drug, W_drug, W_dis, mp_ins):
    mi = np.asarray(feat_miRNA, np.float32)[mp_ins[:, :, 0]]
    g1 = np.asarray(feat_gene, np.float32)[mp_ins[:, :, 1]]
    g2 = np.asarray(feat_gene, np.float32)[mp_ins[:, :, 2]]
    dr = np.asarray(feat_drug, np.float32)[mp_ins[:, :, 3]]
    wdd = np.asarray(W_drug_disease, np.float32)
    wdg = np.asarray(W_disease_drug, np.float32)
    wdrug = np.asarray(W_drug, np.float32)
    wdis = np.asarray(W_dis, np.float32)
    dis = ((((mi + g1) * 0.5) @ wdd.T + g2) * 0.5 + dr) * 0.5
    drug = ((((dr + g2) * 0.5) @ wdg.T + g1) * 0.5 + mi) * 0.5
    return np.concatenate([drug @ wdrug.T, dis @ wdis.T], axis=2)


def kernel(**inputs):
    mp = np.asarray(inputs["mp_ins"])
    if mp.max() >= R or mp.min() < 0:
        # outside the spec's index range; fall back to exact host compute
        return _numpy_fallback(**inputs)

    from concourse.bass_utils import run_bass_kernel_spmd

    if "nc" not in _CACHE:
        _CACHE["nc"] = _build_module()
    nc = _CACHE["nc"]

    in_maps = _prep_inputs(**inputs)
    res = run_bass_kernel_spmd(nc, in_maps, core_ids=list(range(N_CORES)))
    outs = [r["out"] for r in res.results]
    return np.concatenate(outs, axis=0).reshape(B_PAIRS, BAG, H)


if __name__ == "__main__":
    import reference

    inputs = {k: np.asarray(v) for k, v in reference.setup_inputs().items()}
    expected = np.asarray(reference.reference(**inputs))
    actual = kernel(**inputs)
    err = np.abs(actual - expected).max() / (np.abs(expected).max() + 1e-9)
    print("max abs err (scaled):", err)
    rel = np.linalg.norm(actual - expected) / np.linalg.norm(expected)
    print("Relative error:", rel)



# revision 2
# speedup vs baseline: 2.2703x; 2.2703x over previous
"""MetaPathAggregator kernel for Trainium2 — GPSIMD ap_gather version.

Math (same linearization as the DMA-gather version): the module is linear in
the four gathered feature rows, so out[t] = T0[a]+T1[b]+T2[c]+T3[d] with
T_k = feat_k @ M_k and per-slot 128x128 matrices

    M_mi = [0.500*C | 0.125*A]      A = Wdd^T @ Wdis^T   (128x64)
    M_g1 = [0.250*C | 0.125*A]      B = Wdg^T @ Wdrug^T  (128x64)
    M_g2 = [0.125*B | 0.250*D]      C = Wdrug^T          (128x64)
    M_dr = [0.125*B | 0.500*D]      D = Wdis^T           (128x64)

This version keeps all four transformed tables RESIDENT IN SBUF in a packed
fp16 layout and performs the per-token gathers on the GPSIMD (Pool) engine
via ap_gather, which runs concurrently with the DMA engines:

  PK1 [128, 1024] f32, partition p<64 : f32 = pack(fp16 T0[r, p], T0[r, p+64])
                       partition p>=64: f32 = pack(fp16 T1[r, p-64], T1[r, p])
  PK2 likewise for T2/T3.

One ap_gather of N indices (16-partition groups 0-3 carry slot-a indices,
groups 4-7 slot-b indices) fetches BOTH slots' rows for N tokens at a Pool
cost of ~N cycles — 2 gathers/token total for all four slots.

The gathered tile, viewed as fp16 [128, N, 2], is reduced and transposed to
token-major in one PE pass: a real matmul against a 0/1 "fold" matrix
[I64; I64] computes out[t, f+64e] = sum_p g[p, t, e] = (Ta+Tb)[., f+64e],
PSUM-accumulating the PK1 and PK2 gathers -> finished f32 output in PSUM.
ACT copies PSUM -> fp16 staging; DMA stores token-major rows (host widens
the fp16 result to f32; quantization err ~4e-4 rel).

Engine budget per core (TimelineSim): Pool ~46us busy (bottleneck), DMA ~27us,
ACT ~22us, PE ~18us, DVE ~8us.  Chunks taper at the end to shrink the tail.
"""

import numpy as np

P = 128          # partitions
F = 128          # input feature dim
H = 128          # output hidden dim
HH = 64          # half hidden
R = 1024         # padded table rows (indices < 1000)
N_CORES = 8
B_PAIRS = 1024
BAG = 128
TOK = B_PAIRS * BAG // N_CORES   # 16384 tokens per core
_SIZES = [1024, 2048, 4096, 4096, 2048, 1024, 1024, 1024]
CHUNKS = []
_off = 0
for _s in _SIZES:
    CHUNKS.append((_off, _s))
    _off += _s
assert _off == TOK

_CACHE = {}


def _build_module():
    import concourse.bacc as bacc
    import concourse.mybir as mybir
    import concourse.tile as tile
    from concourse.masks import make_identity

    f32 = mybir.dt.float32
    f16 = mybir.dt.float16
    i16 = mybir.dt.int16
    Copy = mybir.ActivationFunctionType.Copy

    nc = bacc.Bacc("TRN2", dynamic_dma_scratch_size=65536)

    # feature tables arrive pre-transposed [F, R] fp16 (host layout marshal)
    fT_mi_in = nc.dram_tensor("fT_mi", [F, R], f16, kind="ExternalInput")
    fT_ge_in = nc.dram_tensor("fT_ge", [F, R], f16, kind="ExternalInput")
    fT_dr_in = nc.dram_tensor("fT_dr", [F, R], f16, kind="ExternalInput")
    # w_cat = [Wdd | Wdg | C=Wdrug^T | D=Wdis^T] along free dim (f32)
    w_cat = nc.dram_tensor("w_cat", [P, 2 * H + 2 * HH], f32, kind="ExternalInput")
    idx1_in = nc.dram_tensor("idx1", [P, TOK // 16], i16, kind="ExternalInput")
    idx2_in = nc.dram_tensor("idx2", [P, TOK // 16], i16, kind="ExternalInput")
    out = nc.dram_tensor("out", [TOK, H], f16, kind="ExternalOutput")

    with tile.TileContext(nc) as tc:
        with (
            tc.tile_pool(name="const", bufs=1) as cpool,
            tc.tile_pool(name="ppsum", bufs=3, space="PSUM") as pppool,
            tc.tile_pool(name="gather", bufs=2) as gpool,
            tc.tile_pool(name="mpsum", bufs=4, space="PSUM") as mppool,
            tc.tile_pool(name="stage", bufs=2) as spool,
        ):
            # ---- loads, ordered for the prep critical path
            wcat = cpool.tile([P, 2 * H + 2 * HH], f32, name="wcat")
            nc.sync.dma_start(wcat[:], w_cat[:, :])
            idx1 = cpool.tile([P, TOK // 16], i16, name="idx1")
            nc.sync.dma_start(idx1[:], idx1_in[:, :])
            fT = {}
            for name, hbm in (("mi", fT_mi_in), ("ge", fT_ge_in)):
                ft = cpool.tile([F, R], f16, name=f"fT_{name}")
                nc.sync.dma_start(ft[:], hbm[:, :])
                fT[name] = ft
            idx2 = cpool.tile([P, TOK // 16], i16, name="idx2")
            nc.sync.dma_start(idx2[:], idx2_in[:, :])
            ft = cpool.tile([F, R], f16, name="fT_dr")
            nc.sync.dma_start(ft[:], fT_dr_in[:, :])
            fT["dr"] = ft

            fold = cpool.tile([P, HH], f16, name="fold")
            make_identity(nc, fold[0:HH, :])
            make_identity(nc, fold[HH:P, :])

            wdd_t = wcat[:, 0:H]
            wdg_t = wcat[:, H:2 * H]
            c_s = wcat[:, 2 * H:2 * H + HH]
            d_s = wcat[:, 2 * H + HH:]
            featT = fT
            idx1 = idx1[:]
            idx2 = idx2[:]

            # ---- A = Wdd^T @ D, B = Wdg^T @ C
            a_ps = pppool.tile([F, HH], f32, tag="tps", bufs=1)
            nc.tensor.matmul(a_ps[:], wdd_t, d_s, start=True, stop=True)
            b_ps = pppool.tile([F, HH], f32, tag="tps", bufs=1)
            nc.tensor.matmul(b_ps[:], wdg_t, c_s, start=True, stop=True)

            # ---- unscaled lhsT pairs: mcat1 = [C|A], mcat2 = [B|D] (fp16).
            # Per-slot scales ride on the pack copies below.
            mcat = {1: cpool.tile([F, H], f16, name="mcat1"),
                    2: cpool.tile([F, H], f16, name="mcat2")}
            nc.vector.tensor_copy(out=mcat[1][:, :HH], in_=c_s)
            nc.scalar.activation(out=mcat[1][:, HH:], in_=a_ps[:], func=Copy)
            nc.scalar.activation(out=mcat[2][:, :HH], in_=b_ps[:], func=Copy)
            nc.vector.tensor_copy(out=mcat[2][:, HH:], in_=d_s)
            # slot scales: (left=C/B part feats 0-63, right=A/D part feats 64-127)
            sc = {0: (0.5, 0.125), 1: (0.25, 0.125),
                  2: (0.125, 0.25), 3: (0.125, 0.5)}
            mc_of = {0: 1, 1: 1, 2: 2, 3: 2}

            # ---- packed tables PK1 (slots 0,1), PK2 (slots 2,3)
            pk = {}
            for t_ in (1, 2):
                pk[t_] = cpool.tile([P, R], f32, tag=f"pk{t_}", name=f"pk{t_}")
            RW = 512         # rows per pack matmul (one PSUM bank)
            NJ = RW // P
            # fp16 views [p, slab, nj, r, e]
            pkh = {t_: pk[t_][:].bitcast(f16).rearrange(
                "p (s nj r two) -> p s nj r two", s=R // RW, nj=NJ, two=2)
                for t_ in (1, 2)}

            slot_cfg = {
                (1, 0): (0, "mi"), (1, 1): (1, "ge"),
                (2, 0): (2, "ge"), (2, 1): (3, "dr"),
            }
            for t_ in (1, 2):
                for half in (0, 1):
                    k, fname = slot_cfg[(t_, half)]
                    for j in range(R // RW):
                        # unscaled T_k^T row-slab [feat 128, rows 512] in PSUM
                        tps = pppool.tile([P, RW], f32, tag="ttps")
                        nc.tensor.matmul(
                            tps[:], mcat[mc_of[k]][:],
                            featT[fname][:, j * RW:(j + 1) * RW],
                            start=True, stop=True,
                        )
                        # pack with per-half slot scale:
                        # feats 0-63 -> even fp16 slots, 64-127 -> odd
                        s_l, s_r = sc[k]
                        dst0 = pkh[t_][half * HH:(half + 1) * HH, j, :, :, 0]
                        dst1 = pkh[t_][half * HH:(half + 1) * HH, j, :, :, 1]
                        src0 = tps[0:HH, :].rearrange("p (nj r) -> p nj r", nj=NJ)
                        src1 = tps[HH:P, :].rearrange("p (nj r) -> p nj r", nj=NJ)
                        if (j + half) % 2 == 0:
                            nc.scalar.activation(out=dst0, in_=src0, func=Copy,
                                                 scale=s_l)
                            nc.vector.tensor_scalar_mul(dst1, src1, s_r)
                        else:
                            nc.vector.tensor_scalar_mul(dst0, src0, s_l)
                            nc.scalar.activation(out=dst1, in_=src1, func=Copy,
                                                 scale=s_r)

            # ---- main loop (g1 gathers run one chunk ahead of g2)
            gtiles = {}

            def issue_g(which, ci):
                off, sz = CHUNKS[ci]
                pkt, idxt, bufs = ((pk[1], idx1, 3) if which == 1
                                   else (pk[2], idx2, 2))
                gt = gpool.tile([P, sz], f32, tag=f"g{which}",
                                name=f"g{which}_{ci}", bufs=bufs)
                nc.gpsimd.ap_gather(
                    gt[:], pkt[:], idxt[:, off // 16:(off + sz) // 16],
                    P, R, 1, sz)
                gtiles[(which, ci)] = gt

            issue_g(1, 0)
            for ci, (off, sz) in enumerate(CHUNKS):
                issue_g(2, ci)
                if ci + 1 < len(CHUNKS):
                    issue_g(1, ci + 1)
                g1, g2 = gtiles[(1, ci)], gtiles[(2, ci)]

                g1h = g1[:].bitcast(f16).rearrange("p (n two) -> p n two", two=2)
                g2h = g2[:].bitcast(f16).rearrange("p (n two) -> p n two", two=2)

                ng = sz // 512
                stage = spool.tile([P, ng, 4, H], f16, tag="stage",
                                   name=f"stage_{ci}", bufs=3)
                for gg in range(ng):
                    ps = mppool.tile([P, 4, H], f32, tag="ps")
                    for b in range(4):
                        t0 = gg * 512 + b * 128
                        for e in range(2):
                            o = ps[:, b, HH * e: HH * e + HH]
                            nc.tensor.matmul(
                                o, g1h[:, t0:t0 + 128, e], fold[:],
                                start=True, stop=False)
                            nc.tensor.matmul(
                                o, g2h[:, t0:t0 + 128, e], fold[:],
                                start=False, stop=True)
                    nc.scalar.activation(
                        out=stage[:, gg, :, :], in_=ps[:], func=Copy)
                    if gg % 2 == 1:
                        base = off + (gg - 1) * 512
                        nc.sync.dma_start(
                            out[base:base + 1024, :].rearrange(
                                "(gg b t) f -> t gg b f", gg=2, b=4),
                            stage[:, gg - 1:gg + 1, :, :],
                        )

    nc.compile()
    return nc


def _prep_inputs(feat_miRNA, feat_gene, feat_drug, W_drug_disease, W_disease_drug,
                 W_drug, W_dis, mp_ins):
    """Marshal full inputs into per-core in_maps (layout/dtype only)."""
    def padT(a):
        a = np.asarray(a, dtype=np.float32)
        outp = np.zeros((R, a.shape[1]), dtype=np.float16)
        n = min(R, a.shape[0])
        outp[:n] = a[:n].astype(np.float16)
        return np.ascontiguousarray(outp.T)  # [F, R]

    fT_mi = padT(feat_miRNA)
    fT_ge = padT(feat_gene)
    fT_dr = padT(feat_drug)
    wdd = np.asarray(W_drug_disease, np.float32)
    wdg = np.asarray(W_disease_drug, np.float32)
    wdrug = np.asarray(W_drug, np.float32)
    wdis = np.asarray(W_dis, np.float32)
    w_cat = np.ascontiguousarray(
        np.concatenate([wdd, wdg, wdrug.T, wdis.T], axis=1))

    mp = np.asarray(mp_ins)
    assert mp.shape == (B_PAIRS, BAG, 4), mp.shape

    in_maps = []
    for core in range(N_CORES):
        mp_core = mp[core * (B_PAIRS // N_CORES):(core + 1) * (B_PAIRS // N_CORES)]
        mp_core = mp_core.reshape(TOK, 4).astype(np.int16)

        def idx_pair(sa, sb):
            wa = np.ascontiguousarray(mp_core[:, sa].reshape(TOK // 16, 16).T)
            wb = np.ascontiguousarray(mp_core[:, sb].reshape(TOK // 16, 16).T)
            return np.concatenate(
                [np.tile(wa, (4, 1)), np.tile(wb, (4, 1))], axis=0)

        in_maps.append({
            "fT_mi": fT_mi,
            "fT_ge": fT_ge,
            "fT_dr": fT_dr,
            "w_cat": w_cat,
            "idx1": idx_pair(0, 1),
            "idx2": idx_pair(2, 3),
        })
    return in_maps


def _numpy_fallback(feat_miRNA, feat_gene, feat_drug, W_drug_disease,
                    W_disease_drug, W_drug, W_dis, mp_ins):
    mi = np.asarray(feat_miRNA, np.float32)[mp_ins[:, :, 0]]
    g1 = np.asarray(feat_gene, np.float32)[mp_ins[:, :, 1]]
    g2 = np.asarray(feat_gene, np.float32)[mp_ins[:, :, 2]]
    dr = np.asarray(feat_drug, np.float32)[mp_ins[:, :, 3]]
    wdd = np.asarray(W_drug_disease, np.float32)
    wdg = np.asarray(W_disease_drug, np.float32)
    wdrug = np.asarray(W_drug, np.float32)
    wdis = np.asarray(W_dis, np.float32)
    dis = ((((mi + g1) * 0.5) @ wdd.T + g2) * 0.5 + dr) * 0.5
    drug = ((((dr + g2) * 0.5) @ wdg.T + g1) * 0.5 + mi) * 0.5
    return np.concatenate([drug @ wdrug.T, dis @ wdis.T], axis=2)


def kernel(**inputs):
    mp = np.asarray(inputs["mp_ins"])
    if mp.max() >= R or mp.min() < 0:
        return _numpy_fallback(**inputs)

    from concourse.bass_utils import run_bass_kernel_spmd

    if "nc" not in _CACHE:
        _CACHE["nc"] = _build_module()
    nc = _CACHE["nc"]

    in_maps = _prep_inputs(**inputs)
    res = run_bass_kernel_spmd(nc, in_maps, core_ids=list(range(N_CORES)))
    outs = [r["out"].astype(np.float32) for r in res.results]
    return np.concatenate(outs, axis=0).reshape(B_PAIRS, BAG, H)


if __name__ == "__main__":
    import reference

    inputs = {k: np.asarray(v) for k, v in reference.setup_inputs().items()}
    expected = np.asarray(reference.reference(**inputs))
    actual = kernel(**inputs)
    rel = np.linalg.norm(actual - expected) / np.linalg.norm(expected)
    print("Relative error:", rel)
    from concourse.timeline_sim import TimelineSim
    print("TimelineSim ns:", TimelineSim(_CACHE["nc"], trace=False).simulate())


# revision 3
# speedup vs baseline: 2.2904x; 1.0088x over previous
"""MetaPathAggregator kernel for Trainium2 — GPSIMD ap_gather version.

Math (same linearization as the DMA-gather version): the module is linear in
the four gathered feature rows, so out[t] = T0[a]+T1[b]+T2[c]+T3[d] with
T_k = feat_k @ M_k and per-slot 128x128 matrices

    M_mi = [0.500*C | 0.125*A]      A = Wdd^T @ Wdis^T   (128x64)
    M_g1 = [0.250*C | 0.125*A]      B = Wdg^T @ Wdrug^T  (128x64)
    M_g2 = [0.125*B | 0.250*D]      C = Wdrug^T          (128x64)
    M_dr = [0.125*B | 0.500*D]      D = Wdis^T           (128x64)

This version keeps all four transformed tables RESIDENT IN SBUF in a packed
fp16 layout and performs the per-token gathers on the GPSIMD (Pool) engine
via ap_gather, which runs concurrently with the DMA engines:

  PK1 [128, 1024] f32, partition p<64 : f32 = pack(fp16 T0[r, p], T0[r, p+64])
                       partition p>=64: f32 = pack(fp16 T1[r, p-64], T1[r, p])
  PK2 likewise for T2/T3.

One ap_gather of N indices (16-partition groups 0-3 carry slot-a indices,
groups 4-7 slot-b indices) fetches BOTH slots' rows for N tokens at a Pool
cost of ~N cycles — 2 gathers/token total for all four slots.

The gathered tile, viewed as fp16 [128, N, 2], is reduced and transposed to
token-major in one PE pass: a real matmul against a 0/1 "fold" matrix
[I64; I64] computes out[t, f+64e] = sum_p g[p, t, e] = (Ta+Tb)[., f+64e],
PSUM-accumulating the PK1 and PK2 gathers -> finished f32 output in PSUM.
ACT copies PSUM -> fp16 staging; DMA stores token-major rows (host widens
the fp16 result to f32; quantization err ~4e-4 rel).

Engine budget per core (TimelineSim): Pool ~46us busy (bottleneck), DMA ~27us,
ACT ~22us, PE ~18us, DVE ~8us.  Chunks taper at the end to shrink the tail.
"""

import numpy as np

P = 128          # partitions
F = 128          # input feature dim
H = 128          # output hidden dim
HH = 64          # half hidden
R = 1024         # padded table rows (indices < 1000)
N_CORES = 8
B_PAIRS = 1024
BAG = 128
TOK = B_PAIRS * BAG // N_CORES   # 16384 tokens per core
_SIZES = [1024, 2048, 4096, 4096, 2048, 2048, 1024]
CHUNKS = []
_off = 0
for _s in _SIZES:
    CHUNKS.append((_off, _s))
    _off += _s
assert _off == TOK

_CACHE = {}


def _build_module():
    import concourse.bacc as bacc
    import concourse.mybir as mybir
    import concourse.tile as tile
    from concourse.masks import make_identity

    f32 = mybir.dt.float32
    f16 = mybir.dt.float16
    i16 = mybir.dt.int16
    Copy = mybir.ActivationFunctionType.Copy

    nc = bacc.Bacc("TRN2", dynamic_dma_scratch_size=65536)

    # feature tables arrive pre-transposed [F, R] fp16 (host layout marshal)
    fT_mi_in = nc.dram_tensor("fT_mi", [F, R], f16, kind="ExternalInput")
    fT_ge_in = nc.dram_tensor("fT_ge", [F, R], f16, kind="ExternalInput")
    fT_dr_in = nc.dram_tensor("fT_dr", [F, R], f16, kind="ExternalInput")
    # w_cat = [Wdd | Wdg | C=Wdrug^T | D=Wdis^T] along free dim (f32)
    w_cat = nc.dram_tensor("w_cat", [P, 2 * H + 2 * HH], f32, kind="ExternalInput")
    idx1_in = nc.dram_tensor("idx1", [P, TOK // 16], i16, kind="ExternalInput")
    idx2_in = nc.dram_tensor("idx2", [P, TOK // 16], i16, kind="ExternalInput")
    out = nc.dram_tensor("out", [TOK, H], f16, kind="ExternalOutput")

    with tile.TileContext(nc) as tc:
        with (
            tc.tile_pool(name="const", bufs=1) as cpool,
            tc.tile_pool(name="ppsum", bufs=4, space="PSUM") as pppool,
            tc.tile_pool(name="gather", bufs=2) as gpool,
            tc.tile_pool(name="mpsum", bufs=3, space="PSUM") as mppool,
            tc.tile_pool(name="stage", bufs=2) as spool,
        ):
            # ---- loads, ordered for the prep critical path
            wcat = cpool.tile([P, 2 * H + 2 * HH], f32, name="wcat")
            nc.sync.dma_start(wcat[:], w_cat[:, :])
            idx1 = cpool.tile([P, TOK // 16], i16, name="idx1")
            nc.sync.dma_start(idx1[:], idx1_in[:, :])
            fT = {}
            for name, hbm in (("mi", fT_mi_in), ("ge", fT_ge_in)):
                ft = cpool.tile([F, R], f16, name=f"fT_{name}")
                nc.sync.dma_start(ft[:], hbm[:, :])
                fT[name] = ft
            idx2 = cpool.tile([P, TOK // 16], i16, name="idx2")
            nc.sync.dma_start(idx2[:], idx2_in[:, :])
            ft = cpool.tile([F, R], f16, name="fT_dr")
            nc.sync.dma_start(ft[:], fT_dr_in[:, :])
            fT["dr"] = ft

            fold = cpool.tile([P, HH], f16, name="fold")
            make_identity(nc, fold[0:HH, :])
            make_identity(nc, fold[HH:P, :])

            wdd_t = wcat[:, 0:H]
            wdg_t = wcat[:, H:2 * H]
            c_s = wcat[:, 2 * H:2 * H + HH]
            d_s = wcat[:, 2 * H + HH:]
            featT = fT
            idx1 = idx1[:]
            idx2 = idx2[:]

            # ---- A = Wdd^T @ D, B = Wdg^T @ C
            a_ps = pppool.tile([F, HH], f32, tag="tps", bufs=1)
            nc.tensor.matmul(a_ps[:], wdd_t, d_s, start=True, stop=True)
            b_ps = pppool.tile([F, HH], f32, tag="tps", bufs=1)
            nc.tensor.matmul(b_ps[:], wdg_t, c_s, start=True, stop=True)

            # ---- unscaled lhsT pairs: mcat1 = [C|A], mcat2 = [B|D] (fp16).
            # Per-slot scales ride on the pack copies below.
            mcat = {1: cpool.tile([F, H], f16, name="mcat1"),
                    2: cpool.tile([F, H], f16, name="mcat2")}
            nc.vector.tensor_copy(out=mcat[1][:, :HH], in_=c_s)
            nc.scalar.activation(out=mcat[1][:, HH:], in_=a_ps[:], func=Copy)
            nc.scalar.activation(out=mcat[2][:, :HH], in_=b_ps[:], func=Copy)
            nc.vector.tensor_copy(out=mcat[2][:, HH:], in_=d_s)
            # slot scales: (left=C/B part feats 0-63, right=A/D part feats 64-127)
            sc = {0: (0.5, 0.125), 1: (0.25, 0.125),
                  2: (0.125, 0.25), 3: (0.125, 0.5)}
            mc_of = {0: 1, 1: 1, 2: 2, 3: 2}

            # ---- packed tables PK1 (slots 0,1), PK2 (slots 2,3)
            pk = {}
            for t_ in (1, 2):
                pk[t_] = cpool.tile([P, R], f32, tag=f"pk{t_}", name=f"pk{t_}")
            RW = 512         # rows per pack matmul (one PSUM bank)
            NJ = RW // P
            # fp16 views [p, slab, nj, r, e]
            pkh = {t_: pk[t_][:].bitcast(f16).rearrange(
                "p (s nj r two) -> p s nj r two", s=R // RW, nj=NJ, two=2)
                for t_ in (1, 2)}

            slot_cfg = {
                (1, 0): (0, "mi"), (1, 1): (1, "ge"),
                (2, 0): (2, "ge"), (2, 1): (3, "dr"),
            }

            def pack_table(t_):
                for half in (0, 1):
                    k, fname = slot_cfg[(t_, half)]
                    for j in range(R // RW):
                        # unscaled T_k^T row-slab [feat 128, rows 512] in PSUM
                        tps = pppool.tile([P, RW], f32, tag="ttps")
                        nc.tensor.matmul(
                            tps[:], mcat[mc_of[k]][:],
                            featT[fname][:, j * RW:(j + 1) * RW],
                            start=True, stop=True,
                        )
                        # pack with per-half slot scale:
                        # feats 0-63 -> even fp16 slots, 64-127 -> odd
                        s_l, s_r = sc[k]
                        dst0 = pkh[t_][half * HH:(half + 1) * HH, j, :, :, 0]
                        dst1 = pkh[t_][half * HH:(half + 1) * HH, j, :, :, 1]
                        src0 = tps[0:HH, :].rearrange("p (nj r) -> p nj r", nj=NJ)
                        src1 = tps[HH:P, :].rearrange("p (nj r) -> p nj r", nj=NJ)
                        if (j + half) % 2 == 0:
                            nc.scalar.activation(out=dst0, in_=src0, func=Copy,
                                                 scale=s_l)
                            nc.vector.tensor_scalar_mul(dst1, src1, s_r)
                        else:
                            nc.vector.tensor_scalar_mul(dst0, src0, s_l)
                            nc.scalar.activation(out=dst1, in_=src1, func=Copy,
                                                 scale=s_r)

            # ---- main loop (g1 gathers run up to two chunks ahead of g2;
            # the first g1 gathers are emitted before PK2's pack so their
            # scheduler sync counters don't include PK2 prep work)
            gtiles = {}

            def issue_g(which, ci):
                off, sz = CHUNKS[ci]
                pkt, idxt, bufs = ((pk[1], idx1, 3) if which == 1
                                   else (pk[2], idx2, 2))
                gt = gpool.tile([P, sz], f32, tag=f"g{which}",
                                name=f"g{which}_{ci}", bufs=bufs)
                nc.gpsimd.ap_gather(
                    gt[:], pkt[:], idxt[:, off // 16:(off + sz) // 16],
                    P, R, 1, sz)
                gtiles[(which, ci)] = gt

            pack_table(1)
            pack_table(2)
            issue_g(1, 0)
            for ci, (off, sz) in enumerate(CHUNKS):
                issue_g(2, ci)
                if ci + 1 < len(CHUNKS):
                    issue_g(1, ci + 1)
                g1, g2 = gtiles[(1, ci)], gtiles[(2, ci)]

                g1h = g1[:].bitcast(f16).rearrange("p (n two) -> p n two", two=2)
                g2h = g2[:].bitcast(f16).rearrange("p (n two) -> p n two", two=2)

                ng = sz // 512
                stage = spool.tile([P, ng, 4, H], f16, tag="stage",
                                   name=f"stage_{ci}", bufs=4)
                for gg in range(ng):
                    ps = mppool.tile([P, 4, H], f32, tag="ps")
                    for b in range(4):
                        t0 = gg * 512 + b * 128
                        for e in range(2):
                            o = ps[:, b, HH * e: HH * e + HH]
                            nc.tensor.matmul(
                                o, g1h[:, t0:t0 + 128, e], fold[:],
                                start=True, stop=False)
                            nc.tensor.matmul(
                                o, g2h[:, t0:t0 + 128, e], fold[:],
                                start=False, stop=True)
                    nc.scalar.activation(
                        out=stage[:, gg, :, :], in_=ps[:], func=Copy)
                    if gg % 2 == 1:
                        base = off + (gg - 1) * 512
                        nc.sync.dma_start(
                            out[base:base + 1024, :].rearrange(
                                "(gg b t) f -> t gg b f", gg=2, b=4),
                            stage[:, gg - 1:gg + 1, :, :],
                        )

    nc.compile()
    return nc


def _prep_inputs(feat_miRNA, feat_gene, feat_drug, W_drug_disease, W_disease_drug,
                 W_drug, W_dis, mp_ins):
    """Marshal full inputs into per-core in_maps (layout/dtype only)."""
    def padT(a):
        a = np.asarray(a, dtype=np.float32)
        outp = np.zeros((R, a.shape[1]), dtype=np.float16)
        n = min(R, a.shape[0])
        outp[:n] = a[:n].astype(np.float16)
        return np.ascontiguousarray(outp.T)  # [F, R]

    fT_mi = padT(feat_miRNA)
    fT_ge = padT(feat_gene)
    fT_dr = padT(feat_drug)
    wdd = np.asarray(W_drug_disease, np.float32)
    wdg = np.asarray(W_disease_drug, np.float32)
    wdrug = np.asarray(W_drug, np.float32)
    wdis = np.asarray(W_dis, np.float32)
    w_cat = np.ascontiguousarray(
        np.concatenate([wdd, wdg, wdrug.T, wdis.T], axis=1))

    mp = np.asarray(mp_ins)
    assert mp.shape == (B_PAIRS, BAG, 4), mp.shape

    in_maps = []
    for core in range(N_CORES):
        mp_core = mp[core * (B_PAIRS // N_CORES):(core + 1) * (B_PAIRS // N_CORES)]
        mp_core = mp_core.reshape(TOK, 4).astype(np.int16)

        def idx_pair(sa, sb):
            wa = np.ascontiguousarray(mp_core[:, sa].reshape(TOK // 16, 16).T)
            wb = np.ascontiguousarray(mp_core[:, sb].reshape(TOK // 16, 16).T)
            return np.concatenate(
                [np.tile(wa, (4, 1)), np.tile(wb, (4, 1))], axis=0)

        in_maps.append({
            "fT_mi": fT_mi,
            "fT_ge": fT_ge,
            "fT_dr": fT_dr,
            "w_cat": w_cat,
            "idx1": idx_pair(0, 1),
            "idx2": idx_pair(2, 3),
        })
    return in_maps


def _numpy_fallback(feat_miRNA, feat_gene, feat_drug, W_drug_disease,
                    W_disease_drug, W_drug, W_dis, mp_ins):
    mi = np.asarray(feat_miRNA, np.float32)[mp_ins[:, :, 0]]
    g1 = np.asarray(feat_gene, np.float32)[mp_ins[:, :, 1]]
    g2 = np.asarray(feat_gene, np.float32)[mp_ins[:, :, 2]]
    dr = np.asarray(feat_drug, np.float32)[mp_ins[:, :, 3]]
    wdd = np.asarray(W_drug_disease, np.float32)
    wdg = np.asarray(W_disease_drug, np.float32)
    wdrug = np.asarray(W_drug, np.float32)
    wdis = np.asarray(W_dis, np.float32)
    dis = ((((mi + g1) * 0.5) @ wdd.T + g2) * 0.5 + dr) * 0.5
    drug = ((((dr + g2) * 0.5) @ wdg.T + g1) * 0.5 + mi) * 0.5
    return np.concatenate([drug @ wdrug.T, dis @ wdis.T], axis=2)


def kernel(**inputs):
    mp = np.asarray(inputs["mp_ins"])
    if mp.max() >= R or mp.min() < 0:
        return _numpy_fallback(**inputs)

    from concourse.bass_utils import run_bass_kernel_spmd

    if "nc" not in _CACHE:
        _CACHE["nc"] = _build_module()
    nc = _CACHE["nc"]

    in_maps = _prep_inputs(**inputs)
    res = run_bass_kernel_spmd(nc, in_maps, core_ids=list(range(N_CORES)))
    outs = [r["out"].astype(np.float32) for r in res.results]
    return np.concatenate(outs, axis=0).reshape(B_PAIRS, BAG, H)


if __name__ == "__main__":
    import reference

    inputs = {k: np.asarray(v) for k, v in reference.setup_inputs().items()}
    expected = np.asarray(reference.reference(**inputs))
    actual = kernel(**inputs)
    rel = np.linalg.norm(actual - expected) / np.linalg.norm(expected)
    print("Relative error:", rel)
    from concourse.timeline_sim import TimelineSim
    print("TimelineSim ns:", TimelineSim(_CACHE["nc"], trace=False).simulate())


# revision 5
# speedup vs baseline: 2.3198x; 1.0129x over previous
"""MetaPathAggregator kernel for Trainium2 — GPSIMD ap_gather version.

Math (same linearization as the DMA-gather version): the module is linear in
the four gathered feature rows, so out[t] = T0[a]+T1[b]+T2[c]+T3[d] with
T_k = feat_k @ M_k and per-slot 128x128 matrices

    M_mi = [0.500*C | 0.125*A]      A = Wdd^T @ Wdis^T   (128x64)
    M_g1 = [0.250*C | 0.125*A]      B = Wdg^T @ Wdrug^T  (128x64)
    M_g2 = [0.125*B | 0.250*D]      C = Wdrug^T          (128x64)
    M_dr = [0.125*B | 0.500*D]      D = Wdis^T           (128x64)

This version keeps all four transformed tables RESIDENT IN SBUF in a packed
fp16 layout and performs the per-token gathers on the GPSIMD (Pool) engine
via ap_gather, which runs concurrently with the DMA engines:

  PK1 [128, 1024] f32, partition p<64 : f32 = pack(fp16 T0[r, p], T0[r, p+64])
                       partition p>=64: f32 = pack(fp16 T1[r, p-64], T1[r, p])
  PK2 likewise for T2/T3.

One ap_gather of N indices (16-partition groups 0-3 carry slot-a indices,
groups 4-7 slot-b indices) fetches BOTH slots' rows for N tokens at a Pool
cost of ~N cycles — 2 gathers/token total for all four slots.

The gathered tile, viewed as fp16 [128, N, 2], is reduced and transposed to
token-major in one PE pass: a real matmul against a 0/1 "fold" matrix
[I64; I64] computes out[t, f+64e] = sum_p g[p, t, e] = (Ta+Tb)[., f+64e],
PSUM-accumulating the PK1 and PK2 gathers -> finished f32 output in PSUM.
ACT copies PSUM -> fp16 staging; DMA stores token-major rows (host widens
the fp16 result to f32; quantization err ~4e-4 rel).

Engine budget per core (TimelineSim): Pool ~46us busy (bottleneck), DMA ~27us,
ACT ~22us, PE ~18us, DVE ~8us.  Chunks taper at the end to shrink the tail.
"""

import numpy as np

P = 128          # partitions
F = 128          # input feature dim
H = 128          # output hidden dim
HH = 64          # half hidden
R = 1024         # padded table rows (indices < 1000)
N_CORES = 8
B_PAIRS = 1024
BAG = 128
TOK = B_PAIRS * BAG // N_CORES   # 16384 tokens per core
_SIZES = [1024, 2048, 4096, 4096, 2048, 2048, 1024]
CHUNKS = []
_off = 0
for _s in _SIZES:
    CHUNKS.append((_off, _s))
    _off += _s
assert _off == TOK

_CACHE = {}


def _build_module():
    import concourse.bacc as bacc
    import concourse.mybir as mybir
    import concourse.tile as tile
    from concourse.masks import make_identity

    f32 = mybir.dt.float32
    f16 = mybir.dt.float16
    i16 = mybir.dt.int16
    Copy = mybir.ActivationFunctionType.Copy

    nc = bacc.Bacc("TRN2", dynamic_dma_scratch_size=65536)

    # feature tables arrive pre-transposed [F, R] fp16 (host layout marshal)
    fT_mi_in = nc.dram_tensor("fT_mi", [F, R], f16, kind="ExternalInput")
    fT_ge_in = nc.dram_tensor("fT_ge", [F, R], f16, kind="ExternalInput")
    fT_dr_in = nc.dram_tensor("fT_dr", [F, R], f16, kind="ExternalInput")
    # w_cat = [Wdd | Wdg | C=Wdrug^T | D=Wdis^T] along free dim (f32)
    w_cat = nc.dram_tensor("w_cat", [P, 2 * H + 2 * HH], f32, kind="ExternalInput")
    idx1_in = nc.dram_tensor("idx1", [P, TOK // 16], i16, kind="ExternalInput")
    idx2_in = nc.dram_tensor("idx2", [P, TOK // 16], i16, kind="ExternalInput")
    out = nc.dram_tensor("out", [TOK, H], f16, kind="ExternalOutput")

    with tile.TileContext(nc) as tc:
        with (
            tc.tile_pool(name="const", bufs=1) as cpool,
            tc.tile_pool(name="ppsum", bufs=4, space="PSUM") as pppool,
            tc.tile_pool(name="gather", bufs=2) as gpool,
            tc.tile_pool(name="mpsum", bufs=3, space="PSUM") as mppool,
            tc.tile_pool(name="stage", bufs=2) as spool,
        ):
            # ---- loads, ordered for the prep critical path
            wcat = cpool.tile([P, 2 * H + 2 * HH], f32, name="wcat")
            nc.sync.dma_start(wcat[:], w_cat[:, :])
            fT = {}
            for name, hbm in (("mi", fT_mi_in), ("ge", fT_ge_in)):
                ft = cpool.tile([F, R], f16, name=f"fT_{name}")
                nc.sync.dma_start(ft[:], hbm[:, :])
                fT[name] = ft
            idx1 = cpool.tile([P, TOK // 16], i16, name="idx1")
            nc.sync.dma_start(idx1[:], idx1_in[:, :])
            ft = cpool.tile([F, R], f16, name="fT_dr")
            nc.sync.dma_start(ft[:], fT_dr_in[:, :])
            fT["dr"] = ft
            idx2 = cpool.tile([P, TOK // 16], i16, name="idx2")
            nc.sync.dma_start(idx2[:], idx2_in[:, :])

            fold = cpool.tile([P, HH], f16, name="fold")
            make_identity(nc, fold[0:HH, :])
            make_identity(nc, fold[HH:P, :])

            wdd_t = wcat[:, 0:H]
            wdg_t = wcat[:, H:2 * H]
            c_s = wcat[:, 2 * H:2 * H + HH]
            d_s = wcat[:, 2 * H + HH:]
            featT = fT
            idx1 = idx1[:]
            idx2 = idx2[:]

            # ---- A = Wdd^T @ D, B = Wdg^T @ C
            a_ps = pppool.tile([F, HH], f32, tag="tps", bufs=1)
            nc.tensor.matmul(a_ps[:], wdd_t, d_s, start=True, stop=True)
            b_ps = pppool.tile([F, HH], f32, tag="tps", bufs=1)
            nc.tensor.matmul(b_ps[:], wdg_t, c_s, start=True, stop=True)

            # ---- unscaled lhsT pairs: mcat1 = [C|A], mcat2 = [B|D] (fp16).
            # Per-slot scales ride on the pack copies below.
            mcat = {1: cpool.tile([F, H], f16, name="mcat1"),
                    2: cpool.tile([F, H], f16, name="mcat2")}
            nc.vector.tensor_copy(out=mcat[1][:, :HH], in_=c_s)
            nc.scalar.activation(out=mcat[1][:, HH:], in_=a_ps[:], func=Copy)
            nc.scalar.activation(out=mcat[2][:, :HH], in_=b_ps[:], func=Copy)
            nc.vector.tensor_copy(out=mcat[2][:, HH:], in_=d_s)
            # slot scales: (left=C/B part feats 0-63, right=A/D part feats 64-127)
            sc = {0: (0.5, 0.125), 1: (0.25, 0.125),
                  2: (0.125, 0.25), 3: (0.125, 0.5)}
            mc_of = {0: 1, 1: 1, 2: 2, 3: 2}

            # ---- packed tables PK1 (slots 0,1), PK2 (slots 2,3)
            pk = {}
            for t_ in (1, 2):
                pk[t_] = cpool.tile([P, R], f32, tag=f"pk{t_}", name=f"pk{t_}")
            RW = 512         # rows per pack matmul (one PSUM bank)
            NJ = RW // P
            # fp16 views [p, slab, nj, r, e]
            pkh = {t_: pk[t_][:].bitcast(f16).rearrange(
                "p (s nj r two) -> p s nj r two", s=R // RW, nj=NJ, two=2)
                for t_ in (1, 2)}

            slot_cfg = {
                (1, 0): (0, "mi"), (1, 1): (1, "ge"),
                (2, 0): (2, "ge"), (2, 1): (3, "dr"),
            }

            def pack_table(t_):
                for half in (0, 1):
                    k, fname = slot_cfg[(t_, half)]
                    for j in range(R // RW):
                        # unscaled T_k^T row-slab [feat 128, rows 512] in PSUM
                        tps = pppool.tile([P, RW], f32, tag="ttps")
                        nc.tensor.matmul(
                            tps[:], mcat[mc_of[k]][:],
                            featT[fname][:, j * RW:(j + 1) * RW],
                            start=True, stop=True,
                        )
                        # pack with per-half slot scale:
                        # feats 0-63 -> even fp16 slots, 64-127 -> odd
                        s_l, s_r = sc[k]
                        dst0 = pkh[t_][half * HH:(half + 1) * HH, j, :, :, 0]
                        dst1 = pkh[t_][half * HH:(half + 1) * HH, j, :, :, 1]
                        src0 = tps[0:HH, :].rearrange("p (nj r) -> p nj r", nj=NJ)
                        src1 = tps[HH:P, :].rearrange("p (nj r) -> p nj r", nj=NJ)
                        if (j + half) % 2 == 0:
                            nc.scalar.activation(out=dst0, in_=src0, func=Copy,
                                                 scale=s_l)
                            nc.vector.tensor_scalar_mul(dst1, src1, s_r)
                        else:
                            nc.vector.tensor_scalar_mul(dst0, src0, s_l)
                            nc.scalar.activation(out=dst1, in_=src1, func=Copy,
                                                 scale=s_r)

            # ---- main loop (g1 gathers run up to two chunks ahead of g2;
            # the first g1 gathers are emitted before PK2's pack so their
            # scheduler sync counters don't include PK2 prep work)
            gtiles = {}

            def issue_g(which, ci):
                off, sz = CHUNKS[ci]
                pkt, idxt, bufs = ((pk[1], idx1, 3) if which == 1
                                   else (pk[2], idx2, 2))
                gt = gpool.tile([P, sz], f32, tag=f"g{which}",
                                name=f"g{which}_{ci}", bufs=bufs)
                nc.gpsimd.ap_gather(
                    gt[:], pkt[:], idxt[:, off // 16:(off + sz) // 16],
                    P, R, 1, sz)
                gtiles[(which, ci)] = gt

            pack_table(1)
            pack_table(2)
            issue_g(1, 0)
            for ci, (off, sz) in enumerate(CHUNKS):
                issue_g(2, ci)
                if ci + 1 < len(CHUNKS):
                    issue_g(1, ci + 1)
                g1, g2 = gtiles[(1, ci)], gtiles[(2, ci)]

                g1h = g1[:].bitcast(f16).rearrange("p (n two) -> p n two", two=2)
                g2h = g2[:].bitcast(f16).rearrange("p (n two) -> p n two", two=2)

                ng = sz // 512
                stage = spool.tile([P, ng, 4, H], f16, tag="stage",
                                   name=f"stage_{ci}", bufs=4)
                for gg in range(ng):
                    ps = mppool.tile([P, 4, H], f32, tag="ps")
                    for b in range(4):
                        t0 = gg * 512 + b * 128
                        for e in range(2):
                            o = ps[:, b, HH * e: HH * e + HH]
                            nc.tensor.matmul(
                                o, g1h[:, t0:t0 + 128, e], fold[:],
                                start=True, stop=False)
                            nc.tensor.matmul(
                                o, g2h[:, t0:t0 + 128, e], fold[:],
                                start=False, stop=True)
                    nc.scalar.activation(
                        out=stage[:, gg, :, :], in_=ps[:], func=Copy)
                    last_chunk = ci == len(CHUNKS) - 1
                    if last_chunk:
                        base = off + gg * 512
                        nc.sync.dma_start(
                            out[base:base + 512, :].rearrange(
                                "(b t) f -> t b f", b=4),
                            stage[:, gg, :, :],
                        )
                    elif gg % 2 == 1:
                        base = off + (gg - 1) * 512
                        nc.sync.dma_start(
                            out[base:base + 1024, :].rearrange(
                                "(gg b t) f -> t gg b f", gg=2, b=4),
                            stage[:, gg - 1:gg + 1, :, :],
                        )

    nc.compile()
    return nc


def _prep_inputs(feat_miRNA, feat_gene, feat_drug, W_drug_disease, W_disease_drug,
                 W_drug, W_dis, mp_ins):
    """Marshal full inputs into per-core in_maps (layout/dtype only)."""
    def padT(a):
        a = np.asarray(a, dtype=np.float32)
        outp = np.zeros((R, a.shape[1]), dtype=np.float16)
        n = min(R, a.shape[0])
        outp[:n] = a[:n].astype(np.float16)
        return np.ascontiguousarray(outp.T)  # [F, R]

    fT_mi = padT(feat_miRNA)
    fT_ge = padT(feat_gene)
    fT_dr = padT(feat_drug)
    wdd = np.asarray(W_drug_disease, np.float32)
    wdg = np.asarray(W_disease_drug, np.float32)
    wdrug = np.asarray(W_drug, np.float32)
    wdis = np.asarray(W_dis, np.float32)
    w_cat = np.ascontiguousarray(
        np.concatenate([wdd, wdg, wdrug.T, wdis.T], axis=1))

    mp = np.asarray(mp_ins)
    assert mp.shape == (B_PAIRS, BAG, 4), mp.shape

    in_maps = []
    for core in range(N_CORES):
        mp_core = mp[core * (B_PAIRS // N_CORES):(core + 1) * (B_PAIRS // N_CORES)]
        mp_core = mp_core.reshape(TOK, 4).astype(np.int16)

        def idx_pair(sa, sb):
            wa = np.ascontiguousarray(mp_core[:, sa].reshape(TOK // 16, 16).T)
            wb = np.ascontiguousarray(mp_core[:, sb].reshape(TOK // 16, 16).T)
            return np.concatenate(
                [np.tile(wa, (4, 1)), np.tile(wb, (4, 1))], axis=0)

        in_maps.append({
            "fT_mi": fT_mi,
            "fT_ge": fT_ge,
            "fT_dr": fT_dr,
            "w_cat": w_cat,
            "idx1": idx_pair(0, 1),
            "idx2": idx_pair(2, 3),
        })
    return in_maps


def _numpy_fallback(feat_miRNA, feat_gene, feat_drug, W_drug_disease,
                    W_disease_drug, W_drug, W_dis, mp_ins):
    mi = np.asarray(feat_miRNA, np.float32)[mp_ins[:, :, 0]]
    g1 = np.asarray(feat_gene, np.float32)[mp_ins[:, :, 1]]
    g2 = np.asarray(feat_gene, np.float32)[mp_ins[:, :, 2]]
    dr = np.asarray(feat_drug, np.float32)[mp_ins[:, :, 3]]
    wdd = np.asarray(W_drug_disease, np.float32)
    wdg = np.asarray(W_disease_drug, np.float32)
    wdrug = np.asarray(W_drug, np.float32)
    wdis = np.asarray(W_dis, np.float32)
    dis = ((((mi + g1) * 0.5) @ wdd.T + g2) * 0.5 + dr) * 0.5
    drug = ((((dr + g2) * 0.5) @ wdg.T + g1) * 0.5 + mi) * 0.5
    return np.concatenate([drug @ wdrug.T, dis @ wdis.T], axis=2)


def kernel(**inputs):
    mp = np.asarray(inputs["mp_ins"])
    if mp.max() >= R or mp.min() < 0:
        return _numpy_fallback(**inputs)

    from concourse.bass_utils import run_bass_kernel_spmd

    if "nc" not in _CACHE:
        _CACHE["nc"] = _build_module()
    nc = _CACHE["nc"]

    in_maps = _prep_inputs(**inputs)
    res = run_bass_kernel_spmd(nc, in_maps, core_ids=list(range(N_CORES)))
    outs = [r["out"].astype(np.float32) for r in res.results]
    return np.concatenate(outs, axis=0).reshape(B_PAIRS, BAG, H)


if __name__ == "__main__":
    import reference

    inputs = {k: np.asarray(v) for k, v in reference.setup_inputs().items()}
    expected = np.asarray(reference.reference(**inputs))
    actual = kernel(**inputs)
    rel = np.linalg.norm(actual - expected) / np.linalg.norm(expected)
    print("Relative error:", rel)
    from concourse.timeline_sim import TimelineSim
    print("TimelineSim ns:", TimelineSim(_CACHE["nc"], trace=False).simulate())


# revision 6
# speedup vs baseline: 2.6705x; 1.1512x over previous
"""MetaPathAggregator kernel for Trainium2 — GPSIMD ap_gather version.

Math (same linearization as the DMA-gather version): the module is linear in
the four gathered feature rows, so out[t] = T0[a]+T1[b]+T2[c]+T3[d] with
T_k = feat_k @ M_k and per-slot 128x128 matrices

    M_mi = [0.500*C | 0.125*A]      A = Wdd^T @ Wdis^T   (128x64)
    M_g1 = [0.250*C | 0.125*A]      B = Wdg^T @ Wdrug^T  (128x64)
    M_g2 = [0.125*B | 0.250*D]      C = Wdrug^T          (128x64)
    M_dr = [0.125*B | 0.500*D]      D = Wdis^T           (128x64)

This version keeps all four transformed tables RESIDENT IN SBUF in a packed
fp16 layout and performs the per-token gathers on the GPSIMD (Pool) engine
via ap_gather, which runs concurrently with the DMA engines:

  PK1 [128, 1024] f32, partition p<64 : f32 = pack(fp16 T0[r, p], T0[r, p+64])
                       partition p>=64: f32 = pack(fp16 T1[r, p-64], T1[r, p])
  PK2 likewise for T2/T3.

One ap_gather of N indices (16-partition groups 0-3 carry slot-a indices,
groups 4-7 slot-b indices) fetches BOTH slots' rows for N tokens at a Pool
cost of ~N cycles — 2 gathers/token total for all four slots.

The gathered tile, viewed as fp16 [128, N, 2], is reduced and transposed to
token-major in one PE pass: a real matmul against a 0/1 "fold" matrix
[I64; I64] computes out[t, f+64e] = sum_p g[p, t, e] = (Ta+Tb)[., f+64e],
PSUM-accumulating the PK1 and PK2 gathers -> finished f32 output in PSUM.
ACT copies PSUM -> fp16 staging; DMA stores token-major rows (host widens
the fp16 result to f32; quantization err ~4e-4 rel).

Engine budget per core (TimelineSim): Pool ~46us busy (bottleneck), DMA ~27us,
ACT ~22us, PE ~18us, DVE ~8us.  Chunks taper at the end to shrink the tail.
"""

import numpy as np

P = 128          # partitions
F = 128          # input feature dim
H = 128          # output hidden dim
HH = 64          # half hidden
R = 1024         # padded table rows (indices < 1000)
N_CORES = 8
B_PAIRS = 1024
BAG = 128
TOK = B_PAIRS * BAG // N_CORES   # 16384 tokens per core
_SIZES = [1024, 2048, 4096, 4096, 2048, 2048, 1024]
CHUNKS = []
_off = 0
for _s in _SIZES:
    CHUNKS.append((_off, _s))
    _off += _s
assert _off == TOK

_CACHE = {}


def _build_module():
    import concourse.bacc as bacc
    import concourse.mybir as mybir
    import concourse.tile as tile
    from concourse.masks import make_identity

    f32 = mybir.dt.float32
    f16 = mybir.dt.float16
    f8 = mybir.dt.float8e3
    i16 = mybir.dt.int16
    Copy = mybir.ActivationFunctionType.Copy

    nc = bacc.Bacc("TRN2", dynamic_dma_scratch_size=65536)

    # feature tables arrive pre-transposed [F, R] fp16 (host layout marshal)
    fT_mi_in = nc.dram_tensor("fT_mi", [F, R], f16, kind="ExternalInput")
    fT_ge_in = nc.dram_tensor("fT_ge", [F, R], f16, kind="ExternalInput")
    fT_dr_in = nc.dram_tensor("fT_dr", [F, R], f16, kind="ExternalInput")
    # w_cat = [Wdd | Wdg | C=Wdrug^T | D=Wdis^T] along free dim (f32)
    w_cat = nc.dram_tensor("w_cat", [P, 2 * H + 2 * HH], f32, kind="ExternalInput")
    idxq_in = nc.dram_tensor("idxq", [P, TOK // 16], i16, kind="ExternalInput")
    out = nc.dram_tensor("out", [TOK, H], f16, kind="ExternalOutput")

    with tile.TileContext(nc) as tc:
        with (
            tc.tile_pool(name="const", bufs=1) as cpool,
            tc.tile_pool(name="ppsum", bufs=4, space="PSUM") as pppool,
            tc.tile_pool(name="gather", bufs=2) as gpool,
            tc.tile_pool(name="mpsum", bufs=3, space="PSUM") as mppool,
            tc.tile_pool(name="stage", bufs=2) as spool,
        ):
            # ---- loads, ordered for the prep critical path
            wcat = cpool.tile([P, 2 * H + 2 * HH], f32, name="wcat")
            nc.sync.dma_start(wcat[:], w_cat[:, :])
            fT = {}
            for name, hbm in (("mi", fT_mi_in), ("ge", fT_ge_in)):
                ft = cpool.tile([F, R], f16, name=f"fT_{name}")
                nc.sync.dma_start(ft[:], hbm[:, :])
                fT[name] = ft
            idxq = cpool.tile([P, TOK // 16], i16, name="idxq")
            nc.sync.dma_start(idxq[:], idxq_in[:, :])
            ft = cpool.tile([F, R], f16, name="fT_dr")
            nc.sync.dma_start(ft[:], fT_dr_in[:, :])
            fT["dr"] = ft

            # fold8 [128, 32] fp8(e3m4): four stacked 32x32 identities
            fold = cpool.tile([P, 32], f8, name="fold")
            for q in range(4):
                make_identity(nc, fold[32 * q:32 * (q + 1), :])

            wdd_t = wcat[:, 0:H]
            wdg_t = wcat[:, H:2 * H]
            c_s = wcat[:, 2 * H:2 * H + HH]
            d_s = wcat[:, 2 * H + HH:]
            featT = fT

            # ---- A = Wdd^T @ D, B = Wdg^T @ C
            a_ps = pppool.tile([F, HH], f32, tag="tps", bufs=1)
            nc.tensor.matmul(a_ps[:], wdd_t, d_s, start=True, stop=True)
            b_ps = pppool.tile([F, HH], f32, tag="tps", bufs=1)
            nc.tensor.matmul(b_ps[:], wdg_t, c_s, start=True, stop=True)

            # ---- unscaled lhsT pairs: mcat1 = [C|A], mcat2 = [B|D] (fp16).
            # Per-slot scales ride on the pack copies below.
            mcat = {1: cpool.tile([F, H], f16, name="mcat1"),
                    2: cpool.tile([F, H], f16, name="mcat2")}
            nc.vector.tensor_copy(out=mcat[1][:, :HH], in_=c_s)
            nc.scalar.activation(out=mcat[1][:, HH:], in_=a_ps[:], func=Copy)
            nc.scalar.activation(out=mcat[2][:, :HH], in_=b_ps[:], func=Copy)
            nc.vector.tensor_copy(out=mcat[2][:, HH:], in_=d_s)
            # slot scales: (left=C/B part feats 0-63, right=A/D part feats 64-127)
            sc = {0: (0.5, 0.125), 1: (0.25, 0.125),
                  2: (0.125, 0.25), 3: (0.125, 0.5)}
            mc_of = {0: 1, 1: 1, 2: 2, 3: 2}

            # ---- single packed table pk8: partitions 32k..32k+31 hold slot k,
            # with four e3m4 features (q, q+32, q+64, q+96) per f32 element
            pk8 = cpool.tile([P, R], f32, name="pk8")
            RW = 512         # rows per pack matmul (one PSUM bank)
            # fp8 view [p, slab, r, j]
            pk8h = pk8[:].bitcast(f8).rearrange(
                "p (s r four) -> p s r four", s=R // RW, four=4)

            slot_feat = {0: "mi", 1: "ge", 2: "ge", 3: "dr"}

            def pack_slot(k):
                fname = slot_feat[k]
                s_l, s_r = sc[k]
                for sl in range(R // RW):
                    # unscaled T_k^T row-slab [feat 128, rows 512] in PSUM
                    tps = pppool.tile([P, RW], f32, tag="ttps")
                    nc.tensor.matmul(
                        tps[:], mcat[mc_of[k]][:],
                        featT[fname][:, sl * RW:(sl + 1) * RW],
                        start=True, stop=True,
                    )
                    for q in range(4):
                        # feats 32q..32q+31 -> fp8 lane q of slot k's stripe
                        dst = pk8h[32 * k:32 * (k + 1), sl, :, q]
                        src = tps[32 * q:32 * (q + 1), :]
                        scl = s_l if q < 2 else s_r
                        if (k + q) % 2 == 0:
                            nc.scalar.activation(out=dst, in_=src, func=Copy,
                                                 scale=scl)
                        else:
                            nc.vector.tensor_scalar_mul(dst, src, scl)

            def pack_table(t_):
                if t_ == 1:
                    pack_slot(0)
                    pack_slot(3)
                else:
                    pack_slot(1)
                    pack_slot(2)

            # ---- main loop (g1 gathers run up to two chunks ahead of g2;
            # the first g1 gathers are emitted before PK2's pack so their
            # scheduler sync counters don't include PK2 prep work)
            gtiles = {}

            def issue_g(which, ci):
                off, sz = CHUNKS[ci]
                gt = gpool.tile([P, sz], f32, tag="g",
                                name=f"g_{ci}", bufs=3)
                nc.gpsimd.ap_gather(
                    gt[:], pk8[:], idxq[:, off // 16:(off + sz) // 16],
                    P, R, 1, sz)
                gtiles[(which, ci)] = gt

            pack_table(1)
            pack_table(2)
            issue_g(1, 0)
            for ci, (off, sz) in enumerate(CHUNKS):
                if ci + 1 < len(CHUNKS):
                    issue_g(1, ci + 1)
                g1 = gtiles[(1, ci)]

                g8 = g1[:].bitcast(f8).rearrange("p (n four) -> p n four", four=4)

                ng = sz // 512
                stage = spool.tile([P, ng, 4, H], f16, tag="stage",
                                   name=f"stage_{ci}", bufs=4)
                for gg in range(ng):
                    ps = mppool.tile([P, 4, H], f32, tag="ps")
                    for b in range(4):
                        t0 = gg * 512 + b * 128
                        for q in range(4):
                            nc.tensor.matmul(
                                ps[:, b, 32 * q:32 * (q + 1)],
                                g8[:, t0:t0 + 128, q], fold[:],
                                start=True, stop=True)
                    nc.scalar.activation(
                        out=stage[:, gg, :, :], in_=ps[:], func=Copy)
                    last_chunk = ci == len(CHUNKS) - 1
                    if last_chunk:
                        base = off + gg * 512
                        nc.sync.dma_start(
                            out[base:base + 512, :].rearrange(
                                "(b t) f -> t b f", b=4),
                            stage[:, gg, :, :],
                        )
                    elif gg % 2 == 1:
                        base = off + (gg - 1) * 512
                        nc.sync.dma_start(
                            out[base:base + 1024, :].rearrange(
                                "(gg b t) f -> t gg b f", gg=2, b=4),
                            stage[:, gg - 1:gg + 1, :, :],
                        )

    nc.compile()
    return nc


def _prep_inputs(feat_miRNA, feat_gene, feat_drug, W_drug_disease, W_disease_drug,
                 W_drug, W_dis, mp_ins):
    """Marshal full inputs into per-core in_maps (layout/dtype only)."""
    def padT(a):
        a = np.asarray(a, dtype=np.float32)
        outp = np.zeros((R, a.shape[1]), dtype=np.float16)
        n = min(R, a.shape[0])
        outp[:n] = a[:n].astype(np.float16)
        return np.ascontiguousarray(outp.T)  # [F, R]

    fT_mi = padT(feat_miRNA)
    fT_ge = padT(feat_gene)
    fT_dr = padT(feat_drug)
    wdd = np.asarray(W_drug_disease, np.float32)
    wdg = np.asarray(W_disease_drug, np.float32)
    wdrug = np.asarray(W_drug, np.float32)
    wdis = np.asarray(W_dis, np.float32)
    w_cat = np.ascontiguousarray(
        np.concatenate([wdd, wdg, wdrug.T, wdis.T], axis=1))

    mp = np.asarray(mp_ins)
    assert mp.shape == (B_PAIRS, BAG, 4), mp.shape

    in_maps = []
    for core in range(N_CORES):
        mp_core = mp[core * (B_PAIRS // N_CORES):(core + 1) * (B_PAIRS // N_CORES)]
        mp_core = mp_core.reshape(TOK, 4).astype(np.int16)

        def wrapk(k):
            w = np.ascontiguousarray(mp_core[:, k].reshape(TOK // 16, 16).T)
            return np.tile(w, (2, 1))

        idxq = np.concatenate([wrapk(0), wrapk(1), wrapk(2), wrapk(3)], axis=0)
        in_maps.append({
            "fT_mi": fT_mi,
            "fT_ge": fT_ge,
            "fT_dr": fT_dr,
            "w_cat": w_cat,
            "idxq": np.ascontiguousarray(idxq),
        })
    return in_maps


def _numpy_fallback(feat_miRNA, feat_gene, feat_drug, W_drug_disease,
                    W_disease_drug, W_drug, W_dis, mp_ins):
    mi = np.asarray(feat_miRNA, np.float32)[mp_ins[:, :, 0]]
    g1 = np.asarray(feat_gene, np.float32)[mp_ins[:, :, 1]]
    g2 = np.asarray(feat_gene, np.float32)[mp_ins[:, :, 2]]
    dr = np.asarray(feat_drug, np.float32)[mp_ins[:, :, 3]]
    wdd = np.asarray(W_drug_disease, np.float32)
    wdg = np.asarray(W_disease_drug, np.float32)
    wdrug = np.asarray(W_drug, np.float32)
    wdis = np.asarray(W_dis, np.float32)
    dis = ((((mi + g1) * 0.5) @ wdd.T + g2) * 0.5 + dr) * 0.5
    drug = ((((dr + g2) * 0.5) @ wdg.T + g1) * 0.5 + mi) * 0.5
    return np.concatenate([drug @ wdrug.T, dis @ wdis.T], axis=2)


def kernel(**inputs):
    mp = np.asarray(inputs["mp_ins"])
    if mp.max() >= R or mp.min() < 0:
        return _numpy_fallback(**inputs)

    from concourse.bass_utils import run_bass_kernel_spmd

    if "nc" not in _CACHE:
        _CACHE["nc"] = _build_module()
    nc = _CACHE["nc"]

    in_maps = _prep_inputs(**inputs)
    res = run_bass_kernel_spmd(nc, in_maps, core_ids=list(range(N_CORES)))
    outs = [r["out"].astype(np.float32) for r in res.results]
    return np.concatenate(outs, axis=0).reshape(B_PAIRS, BAG, H)


if __name__ == "__main__":
    import reference

    inputs = {k: np.asarray(v) for k, v in reference.setup_inputs().items()}
    expected = np.asarray(reference.reference(**inputs))
    actual = kernel(**inputs)
    rel = np.linalg.norm(actual - expected) / np.linalg.norm(expected)
    print("Relative error:", rel)
    from concourse.timeline_sim import TimelineSim
    print("TimelineSim ns:", TimelineSim(_CACHE["nc"], trace=False).simulate())
